# revision 4
# baseline (speedup 1.0000x reference)
import math
import functools

import numpy as np
import jax
import jax.numpy as jnp

# nn_BaseWeeklyBank dims (hardcoded per contract)
N = 400; E = 3200; Fs = 16; Fp = 8
DS = 32; DC = 32; DP = 32; DB = 64
WK, SL = 7, 288
G = WK * SL            # 2016 independent time-slice graphs
M = 8                  # NeuronCores
GB = G // M            # 252 graphs per core
CH = 12                # graphs per scan step on device
NSTEP = GB // CH       # 21


def _gat(x, W, a_src, a_dst, bias, S, D, n):
    """PyG GATConv (concat=False), batched over leading axis.

    Gather/scatter expressed as dense one-hot matmuls (S: [E',n] src one-hot,
    D: [n,E'] dst one-hot) — compiles to plain dot_generals on neuronxcc.
    Softmax max-subtraction skipped: logits are O(1) so exp is safe in fp32.
    """
    B = x.shape[0]
    H, C = a_src.shape
    h = (x @ W).reshape(B, n, H, C)
    al_s = jnp.einsum('bnhc,hc->bnh', h, a_src)
    al_d = jnp.einsum('bnhc,hc->bnh', h, a_dst)
    als_e = jnp.einsum('en,bnh->beh', S, al_s)
    ald_e = jnp.einsum('en,bnh->beh', D.T, al_d)
    e = jax.nn.leaky_relu(als_e + ald_e, 0.2)                   # [B,E',H]
    w = jnp.exp(e)
    denom = jnp.einsum('ne,beh->bnh', D, w)                     # [B,n,H]
    alpha = w * jnp.einsum('en,bnh->beh', D.T, 1.0 / denom)     # [B,E',H]
    hsrc = jnp.einsum('en,bnhc->behc', S, h.reshape(B, n, H, C))
    msg = alpha[..., None] * hsrc                               # [B,E',H,C]
    out = jnp.einsum('ne,behc->bnhc', D, msg)
    return out.mean(axis=2) + bias                              # [B,n,C]


def _shard_fn(gsl, gwd, x_static, profile_flat, S, D, params):
    """Compute H,pred for this core's GB time slices. gsl/gwd: [GB] slice/week ids."""
    (Ws1, as1_src, as1_dst, bs1, Ws2, as2_src, as2_dst, bs2,
     wd_emb, sl_emb, Wc1, bc1, Wc2, bc2,
     Wp1, bp1, Wp2, bp2,
     Wt1, at1_src, at1_dst, bt1, Wt2, at2_src, at2_dst, bt2,
     Wh1, bh1, Wh2, bh2) = params
    relu = jax.nn.relu
    n = N

    # static graph encoder (tiny; recomputed on every core)
    xs = relu(_gat(x_static[None], Ws1, as1_src, as1_dst, bs1, S, D, n))
    xs = relu(_gat(xs, Ws2, as2_src, as2_dst, bs2, S, D, n))[0]      # [n,DS]

    # calendar features for this core's slices
    ta = (2.0 * math.pi / SL) * gsl.astype(jnp.float32)
    wa = (2.0 * math.pi / WK) * gwd.astype(jnp.float32)
    cyc = jnp.stack([jnp.sin(ta), jnp.cos(ta), jnp.sin(wa), jnp.cos(wa)], -1)
    wknd = (gwd >= 5).astype(jnp.float32)[..., None]
    feat = jnp.concatenate([wd_emb[gwd], sl_emb[gsl], cyc, wknd], -1)    # [GB,29]
    cal = relu(feat @ Wc1 + bc1) @ Wc2 + bc2                             # [GB,DC]

    # profile encoder for this core's slices: profile_flat [GB,n,Fp]
    pe = relu(profile_flat @ Wp1 + bp1) @ Wp2 + bp2                      # [GB,n,DP]

    fused = jnp.concatenate([
        jnp.broadcast_to(xs[None], (GB, n, DS)),
        jnp.broadcast_to(cal[:, None, :], (GB, n, DC)),
        pe], -1)                                                          # [GB,n,96]

    chunks = fused.reshape(NSTEP, CH, n, DS + DC + DP)

    def step(carry, ch):
        h = relu(_gat(ch, Wt1, at1_src, at1_dst, bt1, S, D, n))
        h = relu(_gat(h, Wt2, at2_src, at2_dst, bt2, S, D, n))
        return carry, h

    _, outs = jax.lax.scan(step, None, chunks)
    Hh = outs.reshape(GB, n, DB)
    pred = (relu(Hh @ Wh1 + bh1) @ Wh2 + bh2)[..., 0]                    # [GB,n]
    return Hh, pred


@functools.lru_cache(maxsize=1)
def _get_pmap_fn():
    devs = jax.devices()[:M]
    return jax.pmap(_shard_fn, in_axes=(0, 0, None, 0, None, None, None), devices=devs)


def kernel(x_static, profile_feat, edge_index,
           Ws1, as1_src, as1_dst, bs1, Ws2, as2_src, as2_dst, bs2,
           wd_emb, sl_emb, Wc1, bc1, Wc2, bc2,
           Wp1, bp1, Wp2, bp2,
           Wt1, at1_src, at1_dst, bt1, Wt2, at2_src, at2_dst, bt2,
           Wh1, bh1, Wh2, bh2):
    params = (Ws1, as1_src, as1_dst, bs1, Ws2, as2_src, as2_dst, bs2,
              wd_emb, sl_emb, Wc1, bc1, Wc2, bc2,
              Wp1, bp1, Wp2, bp2,
              Wt1, at1_src, at1_dst, bt1, Wt2, at2_src, at2_dst, bt2,
              Wh1, bh1, Wh2, bh2)
    params = jax.tree.map(lambda a: jnp.asarray(np.asarray(a), jnp.float32), params)

    # shard the G=2016 time slices across 8 cores (pure data parallelism)
    wid = np.broadcast_to(np.arange(WK, dtype=np.int32)[:, None], (WK, SL)).reshape(G)
    sid = np.broadcast_to(np.arange(SL, dtype=np.int32)[None, :], (WK, SL)).reshape(G)
    # profile_feat [n,WK,SL,Fp] -> [G,n,Fp]
    pf = np.asarray(profile_feat).transpose(1, 2, 0, 3).reshape(G, N, Fp)

    gsl = sid.reshape(M, GB)
    gwd = wid.reshape(M, GB)
    pfs = pf.reshape(M, GB, N, Fp)

    ei = np.asarray(edge_index)
    loop = np.arange(N, dtype=ei.dtype)
    src = np.concatenate([ei[0], loop])
    dst = np.concatenate([ei[1], loop])
    Ep = src.shape[0]
    S = np.zeros((Ep, N), np.float32); S[np.arange(Ep), src] = 1.0
    D = np.zeros((N, Ep), np.float32); D[dst, np.arange(Ep)] = 1.0

    fn = _get_pmap_fn()
    Hh, pred = fn(jnp.asarray(gsl), jnp.asarray(gwd),
                  jnp.asarray(np.asarray(x_static), jnp.float32),
                  jnp.asarray(pfs, jnp.float32),
                  jnp.asarray(S), jnp.asarray(D),
                  params)
    Hh = np.asarray(Hh).reshape(G, N, DB).reshape(WK, SL, N, DB)
    pred = np.asarray(pred).reshape(G, N).reshape(WK, SL, N)
    return Hh, pred
